# revision 9
# baseline (speedup 1.0000x reference)
"""LlamaMlpWithLora on 8 Trainium2 NeuronCores.

Tensor-parallel over the intermediate dim (11008 padded to 11264 = 8*1408).
Each core computes gate/up/act for its I-shard plus a full-[T,H] partial of
the down projection; the host sums the 8 partials (all-reduce equivalent).
LoRA A factors are replicated; LoRA B factors are sharded with I.
All matmuls run in bf16 with fp32 PSUM accumulation.
"""

import contextlib
import sys

sys.path.insert(0, "/opt/trn_rl_repo")

import numpy as np
import ml_dtypes

T, H, I, R, A = 4096, 4096, 11008, 16, 4
NC_CORES = 8
IP = 11264               # I padded to a multiple of 8*128
IS = IP // NC_CORES      # 1408 per core
NIT = IS // 128          # 11 i-tiles of 128 per core
NKT = H // 128           # 32 contraction tiles over hidden dim
NTB = T // 512           # 8 token blocks of 512
NHB = H // 512           # 8 output-column blocks of 512

_cached = {}             # reps -> compiled program (compile once per process)


def _build_program(reps=1):
    import concourse.bass as bass
    import concourse.tile as tile
    from concourse import bacc, mybir

    bf = mybir.dt.bfloat16
    f32 = mybir.dt.float32
    mult = mybir.AluOpType.mult
    ds = bass.ds
    silu = mybir.ActivationFunctionType.Silu

    nc = bacc.Bacc("TRN2", target_bir_lowering=False, debug=False,
                   num_devices=NC_CORES)

    # DRAM inputs, pre-tiled on host so every DMA slice is contiguous.
    xt = nc.dram_tensor("xt", [NTB * 128, NKT, 512], bf, kind="ExternalInput")
    gw = nc.dram_tensor("gw", [NIT * 128, NKT, 128], bf, kind="ExternalInput")
    uw = nc.dram_tensor("uw", [NIT * 128, NKT, 128], bf, kind="ExternalInput")
    dw = nc.dram_tensor("dw", [NHB * 128, NIT, 512], bf, kind="ExternalInput")
    wagu = nc.dram_tensor("wagu", [128, NKT, 128], bf, kind="ExternalInput")
    gwb = nc.dram_tensor("gwb", [64, NIT, 128], bf, kind="ExternalInput")
    uwb = nc.dram_tensor("uwb", [64, NIT, 128], bf, kind="ExternalInput")
    dwa = nc.dram_tensor("dwa", [128, NIT, 64], bf, kind="ExternalInput")
    dwb = nc.dram_tensor("dwb", [64, NHB, 512], bf, kind="ExternalInput")
    mask = nc.dram_tensor("mask", [64, NTB, 512], f32, kind="ExternalInput")
    out = nc.dram_tensor("out", [T, H], f32, kind="ExternalOutput")

    with tile.TileContext(nc) as tc:
        with (
            tc.tile_pool(name="const", bufs=1) as cpool,
            tc.tile_pool(name="x", bufs=2) as xpool,
            tc.tile_pool(name="w", bufs=5) as wpool,
            tc.tile_pool(name="dwp", bufs=2) as dwpool,
            tc.tile_pool(name="act", bufs=2) as actpool,
            tc.tile_pool(name="xa", bufs=2) as xapool,
            tc.tile_pool(name="tmp", bufs=2) as tmppool,
            tc.tile_pool(name="o", bufs=4) as opool,
            tc.tile_pool(name="psg", bufs=2, space="PSUM") as psg,
            tc.tile_pool(name="psu", bufs=2, space="PSUM") as psu,
            tc.tile_pool(name="psd", bufs=2, space="PSUM") as psd,
            tc.tile_pool(name="psxa", bufs=1, space="PSUM") as psxa,
            tc.tile_pool(name="psxad", bufs=1, space="PSUM") as psxad,
        ):
            wagu_sb = cpool.tile([128, NKT, 128], bf)
            nc.sync.dma_start(wagu_sb[:], wagu[:])
            gwb_sb = cpool.tile([64, NIT, 128], bf)
            nc.sync.dma_start(gwb_sb[:], gwb[:])
            uwb_sb = cpool.tile([64, NIT, 128], bf)
            nc.sync.dma_start(uwb_sb[:], uwb[:])
            dwa_sb = cpool.tile([128, NIT, 64], bf)
            nc.sync.dma_start(dwa_sb[:], dwa[:])
            dwb_sb = cpool.tile([64, NHB, 512], bf)
            nc.sync.dma_start(dwb_sb[:], dwb[:])
            mask_sb = cpool.tile([64, NTB, 512], f32)
            nc.sync.dma_start(mask_sb[:], mask[:])

            # reps>1 repeats the computation on-device (timing builds only)
            loop_ctx = (tc.For_i(0, reps, 1) if reps > 1
                        else contextlib.nullcontext())
            with loop_ctx:
              xt_tiles = {}

              def load_x(tb):
                  t = xpool.tile([128, NKT, 512], bf, tag="x")
                  nc.sync.dma_start(t[:], xt[ds(tb * 128, 128), :, :])
                  xt_tiles[tb] = t

              load_x(0)
              for tb in range(NTB):
                xt_sb = xt_tiles.pop(tb)

                # LoRA A projection for gate (rows 0:64) and up (rows 64:128)
                pxa = psxa.tile([128, 512], f32, tag="pxa")
                for k in range(NKT):
                    nc.tensor.matmul(pxa[:], wagu_sb[:, k, :], xt_sb[:, k, :],
                                     start=(k == 0), stop=(k == NKT - 1))
                xa_g = xapool.tile([64, 512], bf, tag="xag")
                xa_u = xapool.tile([64, 512], bf, tag="xau")
                nc.vector.tensor_tensor(xa_g[:], pxa[0:64, :],
                                        mask_sb[:, tb, :], mult)
                nc.vector.tensor_tensor(xa_u[:], pxa[64:128, :],
                                        mask_sb[:, tb, :], mult)

                act_sb = actpool.tile([128, NIT, 512], bf, tag="act")
                pxad = psxad.tile([64, 512], f32, tag="pxad")
                for io in range(NIT):
                    gw_sb = wpool.tile([128, NKT, 128], bf, tag="w")
                    nc.sync.dma_start(gw_sb[:], gw[ds(io * 128, 128), :, :])
                    pg = psg.tile([128, 512], f32, tag="pg")
                    for k in range(NKT):
                        nc.tensor.matmul(pg[:], gw_sb[:, k, :], xt_sb[:, k, :],
                                         start=(k == 0), stop=False)
                    nc.tensor.matmul(pg[:], gwb_sb[:, io, :], xa_g[:],
                                     start=False, stop=True)

                    uw_sb = wpool.tile([128, NKT, 128], bf, tag="w")
                    nc.sync.dma_start(uw_sb[:], uw[ds(io * 128, 128), :, :])
                    pu = psu.tile([128, 512], f32, tag="pu")
                    for k in range(NKT):
                        nc.tensor.matmul(pu[:], uw_sb[:, k, :], xt_sb[:, k, :],
                                         start=(k == 0), stop=False)
                    nc.tensor.matmul(pu[:], uwb_sb[:, io, :], xa_u[:],
                                     start=False, stop=True)

                    tmp = tmppool.tile([128, 512], f32, tag="tmp")
                    nc.scalar.activation(tmp[:], pg[:], silu)
                    nc.vector.tensor_tensor(act_sb[:, io, :], tmp[:], pu[:],
                                            mult)
                    # down-LoRA A: project act into rank space, sum over io
                    nc.tensor.matmul(pxad[:], dwa_sb[:, io, :],
                                     act_sb[:, io, :],
                                     start=(io == 0), stop=(io == NIT - 1))

                xad = xapool.tile([64, 512], bf, tag="xad")
                nc.vector.tensor_tensor(xad[:], pxad[:], mask_sb[:, tb, :],
                                        mult)

                if tb + 1 < NTB:
                    load_x(tb + 1)   # prefetch next token block during down

                for hb in range(NHB):
                    dw_sb = dwpool.tile([128, NIT, 512], bf, tag="dw")
                    nc.scalar.dma_start(dw_sb[:], dw[ds(hb * 128, 128), :, :])
                    for t4 in range(4):
                        pd = psd.tile([128, 512], f32, tag="pd")
                        for io in range(NIT):
                            nc.tensor.matmul(
                                pd[:],
                                act_sb[:, io, ds(t4 * 128, 128)],
                                dw_sb[:, io, :],
                                start=(io == 0), stop=False)
                        nc.tensor.matmul(pd[:], xad[:, ds(t4 * 128, 128)],
                                         dwb_sb[:, hb, :],
                                         start=False, stop=True)
                        o_sb = opool.tile([128, 512], f32, tag="o")
                        nc.vector.tensor_copy(o_sb[:], pd[:])
                        nc.gpsimd.dma_start(
                            out[ds(tb * 512 + t4 * 128, 128),
                                ds(hb * 512, 512)],
                            o_sb[:])

    nc.compile()
    return nc


def _host_prep(x, gate_w, up_w, down_w, gate_wa, gate_wb, up_wa, up_wb,
               down_wa, down_wb, seg_ids):
    """Transpose/pad/tile all operands; returns per-core input maps."""
    bf16 = ml_dtypes.bfloat16

    # x^T tiled: layout [tb, hp, ho, ti] flattened to [NTB*128, NKT, 512]
    xT = np.ascontiguousarray(x.T)                                  # [H, T]
    xt_t = xT.reshape(NKT, 128, NTB, 512).transpose(2, 1, 0, 3)
    xt_t = np.ascontiguousarray(xt_t.reshape(NTB * 128, NKT, 512).astype(bf16))

    def gu_tiles(w):  # w: [I, H] -> per-core [NIT*128, NKT, 128] (lhsT tiles)
        wT = np.zeros((H, IP), np.float32)
        wT[:, :I] = w.T
        t = wT.reshape(NKT, 128, IP // 128, 128).transpose(2, 1, 0, 3)
        return [np.ascontiguousarray(
            t[c * NIT:(c + 1) * NIT].reshape(NIT * 128, NKT, 128).astype(bf16))
            for c in range(NC_CORES)]

    gw_c = gu_tiles(gate_w)
    uw_c = gu_tiles(up_w)

    # down_w [H, I] -> down_wT [IP, H]; per-core [NHB*128, NIT, 512]
    dwT = np.zeros((IP, H), np.float32)
    dwT[:I, :] = down_w.T
    dw_c = []
    for c in range(NC_CORES):
        s = dwT[c * IS:(c + 1) * IS]                                # [IS, H]
        t = s.reshape(NIT, 128, NHB, 512).transpose(2, 1, 0, 3)
        dw_c.append(np.ascontiguousarray(
            t.reshape(NHB * 128, NIT, 512).astype(bf16)))

    # LoRA A for gate+up, concatenated: [H, 128] -> [128, NKT, 128]
    wa = np.concatenate([gate_wa.transpose(1, 0, 2).reshape(H, A * R),
                         up_wa.transpose(1, 0, 2).reshape(H, A * R)], axis=1)
    wagu_t = np.ascontiguousarray(
        wa.reshape(NKT, 128, 128).transpose(1, 0, 2).astype(bf16))

    def wb_tiles(wb):  # [A, R, I] -> per-core [64, NIT, 128]
        f = np.zeros((A * R, IP), np.float32)
        f[:, :I] = wb.reshape(A * R, I)
        return [np.ascontiguousarray(
            f[:, c * IS:(c + 1) * IS].reshape(64, NIT, 128).astype(bf16))
            for c in range(NC_CORES)]

    gwb_c = wb_tiles(gate_wb)
    uwb_c = wb_tiles(up_wb)

    # down LoRA A [A, I, R] -> [IP, 64] -> per-core [128, NIT, 64]
    dwa_f = np.zeros((IP, A * R), np.float32)
    dwa_f[:I] = down_wa.transpose(1, 0, 2).reshape(I, A * R)
    dwa_c = []
    for c in range(NC_CORES):
        s = dwa_f[c * IS:(c + 1) * IS]                              # [IS, 64]
        dwa_c.append(np.ascontiguousarray(
            s.reshape(NIT, 128, 64).transpose(1, 0, 2).astype(bf16)))

    # down LoRA B [A, R, H] -> [64, NHB, 512] (replicated)
    dwb_t = np.ascontiguousarray(
        down_wb.reshape(A * R, NHB, 512).astype(bf16))

    # adapter mask [64, NTB, 512] fp32 (rows = a*R+r, same for every r)
    m = (seg_ids[None, :] == np.arange(A, dtype=seg_ids.dtype)[:, None])
    mask_t = np.ascontiguousarray(
        np.repeat(m, R, axis=0).reshape(64, NTB, 512).astype(np.float32))

    in_maps = []
    for c in range(NC_CORES):
        in_maps.append({
            "xt": xt_t, "gw": gw_c[c], "uw": uw_c[c], "dw": dw_c[c],
            "wagu": wagu_t, "gwb": gwb_c[c], "uwb": uwb_c[c],
            "dwa": dwa_c[c], "dwb": dwb_t, "mask": mask_t,
        })
    return in_maps


def get_program(reps=1):
    if reps not in _cached:
        _cached[reps] = _build_program(reps)
    return _cached[reps]


def kernel(x, gate_w, up_w, down_w, gate_wa, gate_wb, up_wa, up_wb,
           down_wa, down_wb, seg_ids):
    from concourse.bass_utils import run_bass_kernel_spmd

    nc = get_program()
    in_maps = _host_prep(x, gate_w, up_w, down_w, gate_wa, gate_wb,
                         up_wa, up_wb, down_wa, down_wb, seg_ids)
    res = run_bass_kernel_spmd(nc, in_maps, core_ids=list(range(NC_CORES)))
    acc = np.zeros((T, H), np.float64)
    for c in range(NC_CORES):
        acc += res.results[c]["out"]
    return acc.astype(np.float32)


# revision 10
# speedup vs baseline: 2.4986x; 2.4986x over previous
"""LlamaMlpWithLora on 8 Trainium2 NeuronCores.

Tensor-parallel over the intermediate dim (11008 padded to 11264 = 8*1408).
Each core computes gate/up/act for its I-shard plus a full-[T,H] partial of
the down projection; the host sums the 8 partials (all-reduce equivalent).
LoRA A factors are replicated; LoRA B factors are sharded with I.
All matmuls run in bf16 with fp32 PSUM accumulation.
"""

import contextlib
import sys

sys.path.insert(0, "/opt/trn_rl_repo")

import numpy as np
import ml_dtypes

T, H, I, R, A = 4096, 4096, 11008, 16, 4
NC_CORES = 8
IP = 11264               # I padded to a multiple of 8*128
IS = IP // NC_CORES      # 1408 per core
NIT = IS // 128          # 11 i-tiles of 128 per core
NKT = H // 128           # 32 contraction tiles over hidden dim
NTB = T // 512           # 8 token blocks of 512
NHB = H // 512           # 8 output-column blocks of 512

_cached = {}             # reps -> compiled program (compile once per process)


def _build_program(reps=1):
    import concourse.bass as bass
    import concourse.tile as tile
    from concourse import bacc, mybir

    bf = mybir.dt.bfloat16
    f32 = mybir.dt.float32
    mult = mybir.AluOpType.mult
    ds = bass.ds
    silu = mybir.ActivationFunctionType.Silu

    nc = bacc.Bacc("TRN2", target_bir_lowering=False, debug=False,
                   num_devices=NC_CORES)

    # DRAM inputs, pre-tiled on host so every DMA slice is contiguous.
    xt = nc.dram_tensor("xt", [NTB * 128, NKT, 512], bf, kind="ExternalInput")
    gw = nc.dram_tensor("gw", [NIT * 128, NKT, 128], bf, kind="ExternalInput")
    uw = nc.dram_tensor("uw", [NIT * 128, NKT, 128], bf, kind="ExternalInput")
    dw = nc.dram_tensor("dw", [NHB * 128, NIT, 512], bf, kind="ExternalInput")
    wagu = nc.dram_tensor("wagu", [128, NKT, 128], bf, kind="ExternalInput")
    gwb = nc.dram_tensor("gwb", [64, NIT, 128], bf, kind="ExternalInput")
    uwb = nc.dram_tensor("uwb", [64, NIT, 128], bf, kind="ExternalInput")
    dwa = nc.dram_tensor("dwa", [128, NIT, 64], bf, kind="ExternalInput")
    dwb = nc.dram_tensor("dwb", [64, NHB, 512], bf, kind="ExternalInput")
    mask = nc.dram_tensor("mask", [64, NTB, 512], f32, kind="ExternalInput")
    out = nc.dram_tensor("out", [T, H], f32, kind="ExternalOutput")

    with tile.TileContext(nc) as tc:
        with (
            tc.tile_pool(name="const", bufs=1) as cpool,
            tc.tile_pool(name="x", bufs=2) as xpool,
            tc.tile_pool(name="w", bufs=4) as wpool,
            tc.tile_pool(name="dwp", bufs=2) as dwpool,
            tc.tile_pool(name="act", bufs=2) as actpool,
            tc.tile_pool(name="xa", bufs=2) as xapool,
            tc.tile_pool(name="tmp", bufs=2) as tmppool,
            tc.tile_pool(name="o", bufs=4) as opool,
            tc.tile_pool(name="psg", bufs=2, space="PSUM") as psg,
            tc.tile_pool(name="psu", bufs=2, space="PSUM") as psu,
            tc.tile_pool(name="psd", bufs=2, space="PSUM") as psd,
            tc.tile_pool(name="psxa", bufs=1, space="PSUM") as psxa,
            tc.tile_pool(name="psxad", bufs=1, space="PSUM") as psxad,
        ):
            wagu_sb = cpool.tile([128, NKT, 128], bf)
            nc.sync.dma_start(wagu_sb[:], wagu[:])
            gwb_sb = cpool.tile([64, NIT, 128], bf)
            nc.sync.dma_start(gwb_sb[:], gwb[:])
            uwb_sb = cpool.tile([64, NIT, 128], bf)
            nc.sync.dma_start(uwb_sb[:], uwb[:])
            dwa_sb = cpool.tile([128, NIT, 64], bf)
            nc.sync.dma_start(dwa_sb[:], dwa[:])
            dwb_sb = cpool.tile([64, NHB, 512], bf)
            nc.sync.dma_start(dwb_sb[:], dwb[:])
            mask_sb = cpool.tile([64, NTB, 512], f32)
            nc.sync.dma_start(mask_sb[:], mask[:])

            # reps>1 repeats the computation on-device (timing builds only)
            loop_ctx = (tc.For_i(0, reps, 1) if reps > 1
                        else contextlib.nullcontext())
            with loop_ctx:
              xt_tiles = {}

              def load_x(tb):
                  t = xpool.tile([128, NKT, 512], bf, tag="x")
                  nc.sync.dma_start(t[:], xt[ds(tb * 128, 128), :, :])
                  xt_tiles[tb] = t

              load_x(0)
              for tb in range(NTB):
                xt_sb = xt_tiles.pop(tb)

                # LoRA A projection for gate (rows 0:64) and up (rows 64:128)
                pxa = psxa.tile([128, 512], f32, tag="pxa")
                for k in range(NKT):
                    nc.tensor.matmul(pxa[:], wagu_sb[:, k, :], xt_sb[:, k, :],
                                     start=(k == 0), stop=(k == NKT - 1))
                xa_g = xapool.tile([64, 512], bf, tag="xag")
                xa_u = xapool.tile([64, 512], bf, tag="xau")
                nc.vector.tensor_tensor(xa_g[:], pxa[0:64, :],
                                        mask_sb[:, tb, :], mult)
                nc.vector.tensor_tensor(xa_u[:], pxa[64:128, :],
                                        mask_sb[:, tb, :], mult)

                act_sb = actpool.tile([128, NIT, 512], bf, tag="act")
                pxad = psxad.tile([64, 512], f32, tag="pxad")
                for io in range(NIT):
                    gw_sb = wpool.tile([128, NKT, 128], bf, tag="w")
                    nc.sync.dma_start(gw_sb[:], gw[ds(io * 128, 128), :, :])
                    pg = psg.tile([128, 512], f32, tag="pg")
                    for k in range(NKT):
                        nc.tensor.matmul(pg[:], gw_sb[:, k, :], xt_sb[:, k, :],
                                         start=(k == 0), stop=False)
                    nc.tensor.matmul(pg[:], gwb_sb[:, io, :], xa_g[:],
                                     start=False, stop=True)

                    uw_sb = wpool.tile([128, NKT, 128], bf, tag="w")
                    nc.sync.dma_start(uw_sb[:], uw[ds(io * 128, 128), :, :])
                    pu = psu.tile([128, 512], f32, tag="pu")
                    for k in range(NKT):
                        nc.tensor.matmul(pu[:], uw_sb[:, k, :], xt_sb[:, k, :],
                                         start=(k == 0), stop=False)
                    nc.tensor.matmul(pu[:], uwb_sb[:, io, :], xa_u[:],
                                     start=False, stop=True)

                    tmp = tmppool.tile([128, 512], f32, tag="tmp")
                    nc.scalar.activation(tmp[:], pg[:], silu)
                    nc.vector.tensor_tensor(act_sb[:, io, :], tmp[:], pu[:],
                                            mult)
                    # down-LoRA A: project act into rank space, sum over io
                    nc.tensor.matmul(pxad[:], dwa_sb[:, io, :],
                                     act_sb[:, io, :],
                                     start=(io == 0), stop=(io == NIT - 1))

                xad = xapool.tile([64, 512], bf, tag="xad")
                nc.vector.tensor_tensor(xad[:], pxad[:], mask_sb[:, tb, :],
                                        mult)

                if tb + 1 < NTB:
                    load_x(tb + 1)   # prefetch next token block during down

                for hb in range(NHB):
                    dw_sb = dwpool.tile([128, NIT, 512], bf, tag="dw")
                    nc.scalar.dma_start(dw_sb[:], dw[ds(hb * 128, 128), :, :])
                    for t4 in range(4):
                        pd = psd.tile([128, 512], f32, tag="pd")
                        for io in range(NIT):
                            nc.tensor.matmul(
                                pd[:],
                                act_sb[:, io, ds(t4 * 128, 128)],
                                dw_sb[:, io, :],
                                start=(io == 0), stop=False)
                        nc.tensor.matmul(pd[:], xad[:, ds(t4 * 128, 128)],
                                         dwb_sb[:, hb, :],
                                         start=False, stop=True)
                        o_sb = opool.tile([128, 512], f32, tag="o")
                        nc.vector.tensor_copy(o_sb[:], pd[:])
                        nc.gpsimd.dma_start(
                            out[ds(tb * 512 + t4 * 128, 128),
                                ds(hb * 512, 512)],
                            o_sb[:])

    nc.compile()
    return nc


def _host_prep(x, gate_w, up_w, down_w, gate_wa, gate_wb, up_wa, up_wb,
               down_wa, down_wb, seg_ids):
    """Transpose/pad/tile all operands; returns per-core input maps."""
    bf16 = ml_dtypes.bfloat16

    # x^T tiled: layout [tb, hp, ho, ti] flattened to [NTB*128, NKT, 512]
    xT = np.ascontiguousarray(x.T)                                  # [H, T]
    xt_t = xT.reshape(NKT, 128, NTB, 512).transpose(2, 1, 0, 3)
    xt_t = np.ascontiguousarray(xt_t.reshape(NTB * 128, NKT, 512).astype(bf16))

    def gu_tiles(w):  # w: [I, H] -> per-core [NIT*128, NKT, 128] (lhsT tiles)
        wT = np.zeros((H, IP), np.float32)
        wT[:, :I] = w.T
        t = wT.reshape(NKT, 128, IP // 128, 128).transpose(2, 1, 0, 3)
        return [np.ascontiguousarray(
            t[c * NIT:(c + 1) * NIT].reshape(NIT * 128, NKT, 128).astype(bf16))
            for c in range(NC_CORES)]

    gw_c = gu_tiles(gate_w)
    uw_c = gu_tiles(up_w)

    # down_w [H, I] -> down_wT [IP, H]; per-core [NHB*128, NIT, 512]
    dwT = np.zeros((IP, H), np.float32)
    dwT[:I, :] = down_w.T
    dw_c = []
    for c in range(NC_CORES):
        s = dwT[c * IS:(c + 1) * IS]                                # [IS, H]
        t = s.reshape(NIT, 128, NHB, 512).transpose(2, 1, 0, 3)
        dw_c.append(np.ascontiguousarray(
            t.reshape(NHB * 128, NIT, 512).astype(bf16)))

    # LoRA A for gate+up, concatenated: [H, 128] -> [128, NKT, 128]
    wa = np.concatenate([gate_wa.transpose(1, 0, 2).reshape(H, A * R),
                         up_wa.transpose(1, 0, 2).reshape(H, A * R)], axis=1)
    wagu_t = np.ascontiguousarray(
        wa.reshape(NKT, 128, 128).transpose(1, 0, 2).astype(bf16))

    def wb_tiles(wb):  # [A, R, I] -> per-core [64, NIT, 128]
        f = np.zeros((A * R, IP), np.float32)
        f[:, :I] = wb.reshape(A * R, I)
        return [np.ascontiguousarray(
            f[:, c * IS:(c + 1) * IS].reshape(64, NIT, 128).astype(bf16))
            for c in range(NC_CORES)]

    gwb_c = wb_tiles(gate_wb)
    uwb_c = wb_tiles(up_wb)

    # down LoRA A [A, I, R] -> [IP, 64] -> per-core [128, NIT, 64]
    dwa_f = np.zeros((IP, A * R), np.float32)
    dwa_f[:I] = down_wa.transpose(1, 0, 2).reshape(I, A * R)
    dwa_c = []
    for c in range(NC_CORES):
        s = dwa_f[c * IS:(c + 1) * IS]                              # [IS, 64]
        dwa_c.append(np.ascontiguousarray(
            s.reshape(NIT, 128, 64).transpose(1, 0, 2).astype(bf16)))

    # down LoRA B [A, R, H] -> [64, NHB, 512] (replicated)
    dwb_t = np.ascontiguousarray(
        down_wb.reshape(A * R, NHB, 512).astype(bf16))

    # adapter mask [64, NTB, 512] fp32 (rows = a*R+r, same for every r)
    m = (seg_ids[None, :] == np.arange(A, dtype=seg_ids.dtype)[:, None])
    mask_t = np.ascontiguousarray(
        np.repeat(m, R, axis=0).reshape(64, NTB, 512).astype(np.float32))

    in_maps = []
    for c in range(NC_CORES):
        in_maps.append({
            "xt": xt_t, "gw": gw_c[c], "uw": uw_c[c], "dw": dw_c[c],
            "wagu": wagu_t, "gwb": gwb_c[c], "uwb": uwb_c[c],
            "dwa": dwa_c[c], "dwb": dwb_t, "mask": mask_t,
        })
    return in_maps


def get_program(reps=1):
    if reps not in _cached:
        _cached[reps] = _build_program(reps)
    return _cached[reps]


def kernel(x, gate_w, up_w, down_w, gate_wa, gate_wb, up_wa, up_wb,
           down_wa, down_wb, seg_ids):
    from concourse.bass_utils import run_bass_kernel_spmd

    nc = get_program()
    in_maps = _host_prep(x, gate_w, up_w, down_w, gate_wa, gate_wb,
                         up_wa, up_wb, down_wa, down_wb, seg_ids)
    res = run_bass_kernel_spmd(nc, in_maps, core_ids=list(range(NC_CORES)))
    acc = np.zeros((T, H), np.float64)
    for c in range(NC_CORES):
        acc += res.results[c]["out"]
    return acc.astype(np.float32)


# revision 11
# speedup vs baseline: 3.4649x; 1.3867x over previous
"""LlamaMlpWithLora on 8 Trainium2 NeuronCores.

Tensor-parallel over the intermediate dim (11008 padded to 11264 = 8*1408).
Each core computes gate/up/act for its I-shard plus a full-[T,H] partial of
the down projection; the host sums the 8 partials (all-reduce equivalent).
LoRA A factors are replicated; LoRA B factors are sharded with I.
All matmuls run in bf16 with fp32 PSUM accumulation.
"""

import contextlib
import sys

sys.path.insert(0, "/opt/trn_rl_repo")

import numpy as np
import ml_dtypes

T, H, I, R, A = 4096, 4096, 11008, 16, 4
NC_CORES = 8
IP = 11264               # I padded to a multiple of 8*128
IS = IP // NC_CORES      # 1408 per core
NIT = IS // 128          # 11 i-tiles of 128 per core
NKT = H // 128           # 32 contraction tiles over hidden dim
NTB = T // 512           # 8 token blocks of 512
NHB = H // 512           # 8 output-column blocks of 512

_cached = {}             # reps -> compiled program (compile once per process)


def _build_program(reps=1, wbufs=4, dw_eng='scalar'):
    import concourse.bass as bass
    import concourse.tile as tile
    from concourse import bacc, mybir

    bf = mybir.dt.bfloat16
    f32 = mybir.dt.float32
    mult = mybir.AluOpType.mult
    ds = bass.ds
    silu = mybir.ActivationFunctionType.Silu

    nc = bacc.Bacc("TRN2", target_bir_lowering=False, debug=False,
                   num_devices=NC_CORES)

    # DRAM inputs, pre-tiled on host so every DMA slice is contiguous.
    xt = nc.dram_tensor("xt", [NTB * 128, NKT, 512], bf, kind="ExternalInput")
    gw = nc.dram_tensor("gw", [NIT * 128, NKT, 128], bf, kind="ExternalInput")
    uw = nc.dram_tensor("uw", [NIT * 128, NKT, 128], bf, kind="ExternalInput")
    dw = nc.dram_tensor("dw", [NHB * 128, NIT, 512], bf, kind="ExternalInput")
    wagu = nc.dram_tensor("wagu", [128, NKT, 128], bf, kind="ExternalInput")
    gwb = nc.dram_tensor("gwb", [64, NIT, 128], bf, kind="ExternalInput")
    uwb = nc.dram_tensor("uwb", [64, NIT, 128], bf, kind="ExternalInput")
    dwa = nc.dram_tensor("dwa", [128, NIT, 64], bf, kind="ExternalInput")
    dwb = nc.dram_tensor("dwb", [64, NHB, 512], bf, kind="ExternalInput")
    mask = nc.dram_tensor("mask", [64, NTB, 512], f32, kind="ExternalInput")
    out = nc.dram_tensor("out", [T, H], f32, kind="ExternalOutput")

    with tile.TileContext(nc) as tc:
        with (
            tc.tile_pool(name="const", bufs=1) as cpool,
            tc.tile_pool(name="x", bufs=2) as xpool,
            tc.tile_pool(name="w", bufs=wbufs) as wpool,
            tc.tile_pool(name="dwp", bufs=2) as dwpool,
            tc.tile_pool(name="act", bufs=2) as actpool,
            tc.tile_pool(name="xa", bufs=2) as xapool,
            tc.tile_pool(name="tmp", bufs=2) as tmppool,
            tc.tile_pool(name="o", bufs=4) as opool,
            tc.tile_pool(name="psg", bufs=2, space="PSUM") as psg,
            tc.tile_pool(name="psu", bufs=2, space="PSUM") as psu,
            tc.tile_pool(name="psd", bufs=2, space="PSUM") as psd,
            tc.tile_pool(name="psxa", bufs=1, space="PSUM") as psxa,
            tc.tile_pool(name="psxad", bufs=1, space="PSUM") as psxad,
        ):
            wagu_sb = cpool.tile([128, NKT, 128], bf)
            nc.sync.dma_start(wagu_sb[:], wagu[:])
            gwb_sb = cpool.tile([64, NIT, 128], bf)
            nc.sync.dma_start(gwb_sb[:], gwb[:])
            uwb_sb = cpool.tile([64, NIT, 128], bf)
            nc.sync.dma_start(uwb_sb[:], uwb[:])
            dwa_sb = cpool.tile([128, NIT, 64], bf)
            nc.sync.dma_start(dwa_sb[:], dwa[:])
            dwb_sb = cpool.tile([64, NHB, 512], bf)
            nc.sync.dma_start(dwb_sb[:], dwb[:])
            mask_sb = cpool.tile([64, NTB, 512], f32)
            nc.sync.dma_start(mask_sb[:], mask[:])

            # reps>1 repeats the computation on-device (timing builds only)
            loop_ctx = (tc.For_i(0, reps, 1) if reps > 1
                        else contextlib.nullcontext())
            with loop_ctx:
              xt_tiles = {}

              def load_x(tb):
                  t = xpool.tile([128, NKT, 512], bf, tag="x")
                  nc.sync.dma_start(t[:], xt[ds(tb * 128, 128), :, :])
                  xt_tiles[tb] = t

              load_x(0)
              for tb in range(NTB):
                xt_sb = xt_tiles.pop(tb)

                # LoRA A projection for gate (rows 0:64) and up (rows 64:128)
                pxa = psxa.tile([128, 512], f32, tag="pxa")
                for k in range(NKT):
                    nc.tensor.matmul(pxa[:], wagu_sb[:, k, :], xt_sb[:, k, :],
                                     start=(k == 0), stop=(k == NKT - 1))
                xa_g = xapool.tile([64, 512], bf, tag="xag")
                xa_u = xapool.tile([64, 512], bf, tag="xau")
                nc.vector.tensor_tensor(xa_g[:], pxa[0:64, :],
                                        mask_sb[:, tb, :], mult)
                nc.vector.tensor_tensor(xa_u[:], pxa[64:128, :],
                                        mask_sb[:, tb, :], mult)

                act_sb = actpool.tile([128, NIT, 512], bf, tag="act")
                pxad = psxad.tile([64, 512], f32, tag="pxad")
                for io in range(NIT):
                    gw_sb = wpool.tile([128, NKT, 128], bf, tag="w")
                    nc.sync.dma_start(gw_sb[:], gw[ds(io * 128, 128), :, :])
                    pg = psg.tile([128, 512], f32, tag="pg")
                    for k in range(NKT):
                        nc.tensor.matmul(pg[:], gw_sb[:, k, :], xt_sb[:, k, :],
                                         start=(k == 0), stop=False)
                    nc.tensor.matmul(pg[:], gwb_sb[:, io, :], xa_g[:],
                                     start=False, stop=True)

                    uw_sb = wpool.tile([128, NKT, 128], bf, tag="w")
                    nc.sync.dma_start(uw_sb[:], uw[ds(io * 128, 128), :, :])
                    pu = psu.tile([128, 512], f32, tag="pu")
                    for k in range(NKT):
                        nc.tensor.matmul(pu[:], uw_sb[:, k, :], xt_sb[:, k, :],
                                         start=(k == 0), stop=False)
                    nc.tensor.matmul(pu[:], uwb_sb[:, io, :], xa_u[:],
                                     start=False, stop=True)

                    tmp = tmppool.tile([128, 512], f32, tag="tmp")
                    nc.scalar.activation(tmp[:], pg[:], silu)
                    nc.vector.tensor_tensor(act_sb[:, io, :], tmp[:], pu[:],
                                            mult)
                    # down-LoRA A: project act into rank space, sum over io
                    nc.tensor.matmul(pxad[:], dwa_sb[:, io, :],
                                     act_sb[:, io, :],
                                     start=(io == 0), stop=(io == NIT - 1))

                xad = xapool.tile([64, 512], bf, tag="xad")
                nc.vector.tensor_tensor(xad[:], pxad[:], mask_sb[:, tb, :],
                                        mult)

                if tb + 1 < NTB:
                    load_x(tb + 1)   # prefetch next token block during down

                for hb in range(NHB):
                    dw_sb = dwpool.tile([128, NIT, 512], bf, tag="dw")
                    getattr(nc, dw_eng).dma_start(dw_sb[:], dw[ds(hb * 128, 128), :, :])
                    for t4 in range(4):
                        pd = psd.tile([128, 512], f32, tag="pd")
                        for io in range(NIT):
                            nc.tensor.matmul(
                                pd[:],
                                act_sb[:, io, ds(t4 * 128, 128)],
                                dw_sb[:, io, :],
                                start=(io == 0), stop=False)
                        nc.tensor.matmul(pd[:], xad[:, ds(t4 * 128, 128)],
                                         dwb_sb[:, hb, :],
                                         start=False, stop=True)
                        o_sb = opool.tile([128, 512], f32, tag="o")
                        nc.vector.tensor_copy(o_sb[:], pd[:])
                        nc.gpsimd.dma_start(
                            out[ds(tb * 512 + t4 * 128, 128),
                                ds(hb * 512, 512)],
                            o_sb[:])

    nc.compile()
    return nc


def _host_prep(x, gate_w, up_w, down_w, gate_wa, gate_wb, up_wa, up_wb,
               down_wa, down_wb, seg_ids):
    """Transpose/pad/tile all operands; returns per-core input maps."""
    bf16 = ml_dtypes.bfloat16

    # x^T tiled: layout [tb, hp, ho, ti] flattened to [NTB*128, NKT, 512]
    xT = np.ascontiguousarray(x.T)                                  # [H, T]
    xt_t = xT.reshape(NKT, 128, NTB, 512).transpose(2, 1, 0, 3)
    xt_t = np.ascontiguousarray(xt_t.reshape(NTB * 128, NKT, 512).astype(bf16))

    def gu_tiles(w):  # w: [I, H] -> per-core [NIT*128, NKT, 128] (lhsT tiles)
        wT = np.zeros((H, IP), np.float32)
        wT[:, :I] = w.T
        t = wT.reshape(NKT, 128, IP // 128, 128).transpose(2, 1, 0, 3)
        return [np.ascontiguousarray(
            t[c * NIT:(c + 1) * NIT].reshape(NIT * 128, NKT, 128).astype(bf16))
            for c in range(NC_CORES)]

    gw_c = gu_tiles(gate_w)
    uw_c = gu_tiles(up_w)

    # down_w [H, I] -> down_wT [IP, H]; per-core [NHB*128, NIT, 512]
    dwT = np.zeros((IP, H), np.float32)
    dwT[:I, :] = down_w.T
    dw_c = []
    for c in range(NC_CORES):
        s = dwT[c * IS:(c + 1) * IS]                                # [IS, H]
        t = s.reshape(NIT, 128, NHB, 512).transpose(2, 1, 0, 3)
        dw_c.append(np.ascontiguousarray(
            t.reshape(NHB * 128, NIT, 512).astype(bf16)))

    # LoRA A for gate+up, concatenated: [H, 128] -> [128, NKT, 128]
    wa = np.concatenate([gate_wa.transpose(1, 0, 2).reshape(H, A * R),
                         up_wa.transpose(1, 0, 2).reshape(H, A * R)], axis=1)
    wagu_t = np.ascontiguousarray(
        wa.reshape(NKT, 128, 128).transpose(1, 0, 2).astype(bf16))

    def wb_tiles(wb):  # [A, R, I] -> per-core [64, NIT, 128]
        f = np.zeros((A * R, IP), np.float32)
        f[:, :I] = wb.reshape(A * R, I)
        return [np.ascontiguousarray(
            f[:, c * IS:(c + 1) * IS].reshape(64, NIT, 128).astype(bf16))
            for c in range(NC_CORES)]

    gwb_c = wb_tiles(gate_wb)
    uwb_c = wb_tiles(up_wb)

    # down LoRA A [A, I, R] -> [IP, 64] -> per-core [128, NIT, 64]
    dwa_f = np.zeros((IP, A * R), np.float32)
    dwa_f[:I] = down_wa.transpose(1, 0, 2).reshape(I, A * R)
    dwa_c = []
    for c in range(NC_CORES):
        s = dwa_f[c * IS:(c + 1) * IS]                              # [IS, 64]
        dwa_c.append(np.ascontiguousarray(
            s.reshape(NIT, 128, 64).transpose(1, 0, 2).astype(bf16)))

    # down LoRA B [A, R, H] -> [64, NHB, 512] (replicated)
    dwb_t = np.ascontiguousarray(
        down_wb.reshape(A * R, NHB, 512).astype(bf16))

    # adapter mask [64, NTB, 512] fp32 (rows = a*R+r, same for every r)
    m = (seg_ids[None, :] == np.arange(A, dtype=seg_ids.dtype)[:, None])
    mask_t = np.ascontiguousarray(
        np.repeat(m, R, axis=0).reshape(64, NTB, 512).astype(np.float32))

    in_maps = []
    for c in range(NC_CORES):
        in_maps.append({
            "xt": xt_t, "gw": gw_c[c], "uw": uw_c[c], "dw": dw_c[c],
            "wagu": wagu_t, "gwb": gwb_c[c], "uwb": uwb_c[c],
            "dwa": dwa_c[c], "dwb": dwb_t, "mask": mask_t,
        })
    return in_maps


def get_program(reps=1, **kw):
    key = (reps, tuple(sorted(kw.items())))
    if key not in _cached:
        _cached[key] = _build_program(reps, **kw)
    return _cached[key]


def kernel(x, gate_w, up_w, down_w, gate_wa, gate_wb, up_wa, up_wb,
           down_wa, down_wb, seg_ids):
    from concourse.bass_utils import run_bass_kernel_spmd

    nc = get_program()
    in_maps = _host_prep(x, gate_w, up_w, down_w, gate_wa, gate_wb,
                         up_wa, up_wb, down_wa, down_wb, seg_ids)
    res = run_bass_kernel_spmd(nc, in_maps, core_ids=list(range(NC_CORES)))
    acc = np.zeros((T, H), np.float64)
    for c in range(NC_CORES):
        acc += res.results[c]["out"]
    return acc.astype(np.float32)
